# revision 3
# baseline (speedup 1.0000x reference)
"""Trainium2 Bass kernel for the AbstractQCP residual operator F @ W.

Math (reference):
    v = y - s; mask = (v >= 0)
    dx = wx; dy = mask*wy; dt = wt        (W = [wx; wy; wt], (n+m+1, K))
    o1 = P@dx + A.T@dy + q dt             (n, K)
    o2 = b dt - A@dx                      (m, K)
    o3 = (x.T P x) dt - (q + 2 P x)@dx - b@dy
    F  = [o1; o2 + (1-mask)*wy; o3]       (since dx==wx, dt==wt the -dPi+W
                                           residual cancels on the n/t blocks)

Sharding across 8 NeuronCores (pure SPMD, no device collectives):
  core i owns output rows: o1[512i:512(i+1)], o2[1024i:1024(i+1)], and a
  partial of o3 (host sums the 8 (1,256) partials).
  GEMM1: lhsT_B = [P[:,cols_i]; A[:,cols_i]; q_i] (12289+pad, 512) -- P
  symmetric so P[:,cols] == P[rows,:].T.  rhs = [W | e] with e=[x;0;0]
  (257 cols) so column 256 of the GEMM1 result is P_i @ x for free.
  GEMM2: lhsT_C = [-A[rows_i,:].T; b_i] (4097+pad, 1024), rhs = n-block
  rows of W plus the wt row.
  All matmul operands bf16 (host-cast), accumulation fp32 in PSUM.
"""

import os
import numpy as np
import ml_dtypes
from contextlib import ExitStack

BF = ml_dtypes.bfloat16

N, M, KP = 4096, 8192, 256
NC = 8
NS, MS = N // NC, M // NC          # 512, 1024
F = KP + 1                         # 257: probes + aug column
KT1, KT2 = 97, 33                  # contraction tiles (128 rows each)
R1, R2 = KT1 * 128, KT2 * 128      # 12416, 4224 (zero-padded)

_NC_CACHE = None


def _build_nc():
    from concourse import bacc, tile, mybir
    from concourse.alu_op_type import AluOpType as op

    dtb = mybir.dt.bfloat16
    dtf = mybir.dt.float32

    nc = bacc.Bacc("TRN2", target_bir_lowering=False, debug=False)

    def din(name, shape, dt):
        return nc.dram_tensor(name, list(shape), dt, kind="ExternalInput").ap()

    bt = din("bt", (R1, NS), dtb)      # GEMM1 lhsT, column-sharded [P;A;q]
    ct = din("ct", (R2, MS), dtb)      # GEMM2 lhsT, [-A_rows.T; b]
    wa = din("wa", (R1, F), dtb)       # [W | e], replicated
    yt = din("yt", (128, 64), dtf)     # y in (partition, m-tile) layout
    st = din("st", (128, 64), dtf)
    yto = din("yto", (128, 8), dtf)    # own m-shard slices of yt/st
    sto = din("sto", (128, 8), dtf)
    nq = din("nq", (128, 4), dtf)      # -q_i
    xv = din("xv", (128, 4), dtb)      # x_i
    nb = din("nb", (128, 8), dtb)      # -b_i
    wown = din("wown", (MS, F), dtb)   # own wy rows (with aug col = 0)
    xw = din("xw", (NS, F), dtb)       # own wx rows (with aug col = x_i)
    out = nc.dram_tensor("out", [NS + MS + 1, KP], dtf, kind="ExternalOutput").ap()

    COPY = mybir.ActivationFunctionType.Copy

    with tile.TileContext(nc) as tc, ExitStack() as ctx:
        dpool = ctx.enter_context(tc.tile_pool(name="d", bufs=1))
        wpool = ctx.enter_context(tc.tile_pool(name="w", bufs=12))
        cpool = ctx.enter_context(tc.tile_pool(name="c", bufs=6))
        spool = ctx.enter_context(tc.tile_pool(name="s", bufs=1))
        opool = ctx.enter_context(tc.tile_pool(name="o", bufs=6))
        pspool = ctx.enter_context(tc.tile_pool(name="ps", bufs=8, space="PSUM"))

        # --- small vectors + masks -------------------------------------
        ytb = spool.tile((128, 64), dtf, tag="ytb")
        nc.sync.dma_start(ytb, yt)
        stb = spool.tile((128, 64), dtf, tag="stb")
        nc.sync.dma_start(stb, st)
        v = spool.tile((128, 64), dtf, tag="v")
        nc.vector.tensor_sub(v, ytb, stb)
        mask = spool.tile((128, 64), dtf, tag="mask")
        nc.vector.tensor_scalar(mask, v, 0.0, None, op.is_ge)

        ytob = spool.tile((128, 8), dtf, tag="ytob")
        nc.sync.dma_start(ytob, yto)
        stob = spool.tile((128, 8), dtf, tag="stob")
        nc.sync.dma_start(stob, sto)
        vo = spool.tile((128, 8), dtf, tag="vo")
        nc.vector.tensor_sub(vo, ytob, stob)
        masko = spool.tile((128, 8), dtf, tag="masko")
        nc.vector.tensor_scalar(masko, vo, 0.0, None, op.is_ge)
        umo = spool.tile((128, 8), dtf, tag="umo")
        nc.vector.tensor_scalar(umo, masko, -1.0, 1.0, op.mult, op.add)

        nqb = spool.tile((128, 4), dtf, tag="nqb")
        nc.sync.dma_start(nqb, nq)
        xvb = spool.tile((128, 4), dtb, tag="xvb")
        nc.sync.dma_start(xvb, xv)
        nbb = spool.tile((128, 8), dtb, tag="nbb")
        nc.sync.dma_start(nbb, nb)

        # --- resident rhs tiles ---------------------------------------
        d = []
        for k in range(KT1):
            t = dpool.tile((128, F), dtb, tag=f"d{k}")
            nc.sync.dma_start(t, wa[k * 128:(k + 1) * 128, :])
            d.append(t)
        # masked copies of the m-block tiles (dy = mask * wy)
        dm = []
        for j in range(64):
            t = dpool.tile((128, F), dtb, tag=f"dm{j}")
            nc.vector.tensor_scalar_mul(t, d[32 + j], mask[:, j:j + 1])
            dm.append(t)

        wo, wm = [], []
        for t_i in range(8):
            a = spool.tile((128, F), dtb, tag=f"wo{t_i}")
            nc.sync.dma_start(a, wown[t_i * 128:(t_i + 1) * 128, :])
            wo.append(a)
            mt = spool.tile((128, F), dtb, tag=f"wm{t_i}")
            nc.vector.tensor_scalar_mul(mt, a, masko[:, t_i:t_i + 1])
            wm.append(mt)
        xwt = []
        for j in range(4):
            a = spool.tile((128, F), dtb, tag=f"xw{j}")
            nc.sync.dma_start(a, xw[j * 128:(j + 1) * 128, :])
            xwt.append(a)

        def rhs1(k):
            return d[k] if (k < 32 or k == 96) else dm[k - 32]

        # --- GEMM1: o1_i (+ Px in col 256) ----------------------------
        ps1 = [pspool.tile((128, F), dtf, tag="ps", name=f"ps1_{m}") for m in range(4)]
        for k in range(KT1):
            btt = wpool.tile((128, NS), dtb, tag="bt")
            nc.sync.dma_start(btt, bt[k * 128:(k + 1) * 128, :])
            for m in range(4):
                nc.tensor.matmul(ps1[m], btt[:, m * 128:(m + 1) * 128], rhs1(k),
                                 start=(k == 0), stop=(k == KT1 - 1))

        px = spool.tile((128, 4), dtb, tag="px")
        cf = spool.tile((128, 4), dtb, tag="cf")
        for m in range(4):
            ot = opool.tile((128, KP), dtf, tag="o")
            nc.vector.tensor_copy(ot, ps1[m][:, 0:KP])
            nc.sync.dma_start(out[m * 128:(m + 1) * 128, :], ot)
            nc.vector.tensor_copy(px[:, m:m + 1], ps1[m][:, KP:KP + 1])
            # cf = -(q + 2 Px) = (Px * -2) + (-q)
            nc.vector.scalar_tensor_tensor(cf[:, m:m + 1], ps1[m][:, KP:KP + 1],
                                           -2.0, nqb[:, m:m + 1],
                                           op.mult, op.add)

        # --- GEMM2: o2pre_i = -A_i@dx + b_i dt ------------------------
        ps2 = [pspool.tile((128, F), dtf, tag="ps", name=f"ps2_{t}") for t in range(8)]
        for k in range(KT2):
            ctt = cpool.tile((128, MS), dtb, tag="ct")
            nc.sync.dma_start(ctt, ct[k * 128:(k + 1) * 128, :])
            rhs = d[k] if k < 32 else d[96]
            for t_i in range(8):
                nc.tensor.matmul(ps2[t_i], ctt[:, t_i * 128:(t_i + 1) * 128], rhs,
                                 start=(k == 0), stop=(k == KT2 - 1))
        for t_i in range(8):
            ft = opool.tile((128, KP), dtf, tag="o")
            # f2 = (wy * (1-mask)) + o2pre
            nc.vector.scalar_tensor_tensor(ft, wo[t_i][:, 0:KP], umo[:, t_i:t_i + 1],
                                           ps2[t_i][:, 0:KP], op.mult, op.add)
            nc.sync.dma_start(out[NS + t_i * 128:NS + (t_i + 1) * 128, :], ft)

        # --- o3 partial ----------------------------------------------
        pso3 = pspool.tile((1, F), dtf, tag="ps")
        for j in range(4):
            nc.tensor.matmul(pso3, cf[:, j:j + 1], xwt[j],
                             start=(j == 0), stop=False)
        for t_i in range(8):
            nc.tensor.matmul(pso3, nbb[:, t_i:t_i + 1], wm[t_i],
                             start=False, stop=(t_i == 7))
        psxx = pspool.tile((1, 1), dtf, tag="ps")
        for j in range(4):
            nc.tensor.matmul(psxx, px[:, j:j + 1], xvb[:, j:j + 1],
                             start=(j == 0), stop=(j == 3))
        o3t = opool.tile((1, KP), dtf, tag="o3t")
        nc.vector.tensor_scalar_mul(o3t, d[96][0:1, 0:KP], psxx[0:1, 0:1])
        o3f = opool.tile((1, KP), dtf, tag="o3f")
        nc.vector.tensor_add(o3f, o3t, pso3[0:1, 0:KP])
        nc.sync.dma_start(out[NS + MS:NS + MS + 1, :], o3f)

    nc.compile()
    return nc


def _get_nc():
    global _NC_CACHE
    if _NC_CACHE is None:
        _NC_CACHE = _build_nc()
    return _NC_CACHE


def _prep_in_maps(P, A, q, b, x, y, s, W):
    P = np.asarray(P, np.float32)
    A = np.asarray(A, np.float32)
    q = np.asarray(q, np.float32)
    b = np.asarray(b, np.float32)
    x = np.asarray(x, np.float32)
    y = np.asarray(y, np.float32)
    s = np.asarray(s, np.float32)
    W = np.asarray(W, np.float32)

    Pb, Ab = P.astype(BF), A.astype(BF)
    qb, bb, xb, Wb = q.astype(BF), b.astype(BF), x.astype(BF), W.astype(BF)

    wa = np.zeros((R1, F), BF)
    wa[:N + M + 1, :KP] = Wb
    wa[:N, KP] = xb
    yt = np.ascontiguousarray(y.reshape(64, 128).T)
    st_ = np.ascontiguousarray(s.reshape(64, 128).T)

    in_maps = []
    for i in range(NC):
        ncol = slice(i * NS, (i + 1) * NS)
        mrow = slice(i * MS, (i + 1) * MS)
        bt = np.zeros((R1, NS), BF)
        bt[:N] = Pb[:, ncol]
        bt[N:N + M] = Ab[:, ncol]
        bt[N + M] = qb[ncol]
        ct = np.zeros((R2, MS), BF)
        ct[:N] = (-A[mrow].T).astype(BF)
        ct[N] = bb[mrow]
        in_maps.append(dict(
            bt=bt, ct=ct, wa=wa, yt=yt, st=st_,
            yto=np.ascontiguousarray(yt[:, 8 * i:8 * i + 8]),
            sto=np.ascontiguousarray(st_[:, 8 * i:8 * i + 8]),
            nq=np.ascontiguousarray((-q[ncol]).reshape(4, 128).T),
            xv=np.ascontiguousarray(x[ncol].reshape(4, 128).T.astype(BF)),
            nb=np.ascontiguousarray((-b[mrow]).reshape(8, 128).T.astype(BF)),
            wown=np.ascontiguousarray(wa[N + i * MS:N + (i + 1) * MS]),
            xw=np.ascontiguousarray(wa[i * NS:(i + 1) * NS]),
        ))
    return in_maps


def _assemble(results):
    Fo = np.empty((N + M + 1, KP), np.float32)
    o3 = np.zeros((KP,), np.float32)
    for i in range(NC):
        o = np.asarray(results[i]["out"], np.float32)
        Fo[i * NS:(i + 1) * NS] = o[:NS]
        Fo[N + i * MS:N + (i + 1) * MS] = o[NS:NS + MS]
        o3 += o[NS + MS]
    Fo[N + M] = o3
    return Fo


def _run_sharded(inputs, trace=False, trace_kwargs=None):
    from concourse import bass_utils
    nc = _get_nc()
    in_maps = _prep_in_maps(**inputs)
    res = bass_utils.run_bass_kernel_spmd(
        nc, in_maps, core_ids=list(range(NC)), trace=trace,
        **(trace_kwargs or {}))
    return _assemble(res.results), res


def kernel(**inputs) -> np.ndarray:
    out, _ = _run_sharded(inputs, trace=False)
    return out


# revision 4
# speedup vs baseline: 1.8524x; 1.8524x over previous
"""Trainium2 Bass kernel for the AbstractQCP residual operator F @ W.

Math (reference):
    v = y - s; mask = (v >= 0)
    dx = wx; dy = mask*wy; dt = wt        (W = [wx; wy; wt], (n+m+1, K))
    o1 = P@dx + A.T@dy + q dt             (n, K)
    o2 = b dt - A@dx                      (m, K)
    o3 = (x.T P x) dt - (q + 2 P x)@dx - b@dy
    F  = [o1; o2 + (1-mask)*wy; o3]       (since dx==wx, dt==wt the -dPi+W
                                           residual cancels on the n/t blocks)

Sharding across 8 NeuronCores (pure SPMD, no device collectives):
  core i owns output rows: o1[512i:512(i+1)], o2[1024i:1024(i+1)], and a
  partial of o3 (host sums the 8 (1,256) partials).
  GEMM1: lhsT_B = [P[:,cols_i]; A[:,cols_i]; q_i] (12289+pad, 512) -- P
  symmetric so P[:,cols] == P[rows,:].T.  rhs = [W | e] with e=[x;0;0]
  (257 cols) so column 256 of the GEMM1 result is P_i @ x for free.
  GEMM2: lhsT_C = [-A[rows_i,:].T; b_i] (4097+pad, 1024), rhs = n-block
  rows of W plus the wt row.
  All matmul operands bf16 (host-cast), accumulation fp32 in PSUM.

All streamed operands are staged in DRAM K-tile-transposed -- shape
(128, ktiles*free) with element (p, k*free+c) = orig(k*128+p, c) -- so a
single DMA moves several K-tiles with >=4KB contiguous per partition.
"""

import os
import numpy as np
import ml_dtypes
from contextlib import ExitStack

BF = ml_dtypes.bfloat16

N, M, KP = 4096, 8192, 256
NC = 8
NS, MS = N // NC, M // NC          # 512, 1024
F = KP + 1                         # 257: probes + aug column
KT1, KT2 = 97, 33                  # contraction tiles (128 rows each)
R1, R2 = KT1 * 128, KT2 * 128      # 12416, 4224 (zero-padded)

G1 = 8     # wa / bt K-tiles per DMA group
G2 = 4     # ct K-tiles per DMA group

_NC_CACHE = None


def _kt(a, ktiles, free):
    """(ktiles*128, free) row-major -> (128, ktiles*free) K-tile-transposed."""
    return np.ascontiguousarray(
        a.reshape(ktiles, 128, free).transpose(1, 0, 2).reshape(128, ktiles * free))


def _build_nc():
    from concourse import bacc, tile, mybir
    from concourse.alu_op_type import AluOpType as op

    dtb = mybir.dt.bfloat16
    dtf = mybir.dt.float32

    nc = bacc.Bacc("TRN2", target_bir_lowering=False, debug=False)

    def din(name, shape, dt):
        return nc.dram_tensor(name, list(shape), dt, kind="ExternalInput").ap()

    bt = din("bt", (128, KT1 * NS), dtb)    # GEMM1 lhsT, K-tile-transposed
    ct = din("ct", (128, KT2 * MS), dtb)    # GEMM2 lhsT, K-tile-transposed
    wa = din("wa", (128, KT1 * F), dtb)     # [W | e] rhs, K-tile-transposed
    yt = din("yt", (128, 64), dtf)          # y in (partition, m-tile) layout
    st = din("st", (128, 64), dtf)
    yto = din("yto", (128, 8), dtf)         # own m-shard slices of yt/st
    sto = din("sto", (128, 8), dtf)
    nq = din("nq", (128, 4), dtf)           # -q_i
    xv = din("xv", (128, 4), dtb)           # x_i
    nb = din("nb", (128, 8), dtb)           # -b_i
    wown = din("wown", (128, 8 * F), dtb)   # own wy rows, K-tile-transposed
    xw = din("xw", (128, 4 * F), dtb)       # own wx rows, K-tile-transposed
    out = nc.dram_tensor("out", [NS + MS + 1, KP], dtf, kind="ExternalOutput").ap()

    NG1 = (KT1 + G1 - 1) // G1              # 13 groups for wa/bt (last = 1 tile)
    NG2 = (KT2 + G2 - 1) // G2              # 9 groups for ct (last = 1 tile)

    with tile.TileContext(nc) as tc, ExitStack() as ctx:
        dpool = ctx.enter_context(tc.tile_pool(name="d", bufs=1))
        wpool = ctx.enter_context(tc.tile_pool(name="w", bufs=3))
        cpool = ctx.enter_context(tc.tile_pool(name="c", bufs=3))
        spool = ctx.enter_context(tc.tile_pool(name="s", bufs=1))
        opool = ctx.enter_context(tc.tile_pool(name="o", bufs=6))
        pspool = ctx.enter_context(tc.tile_pool(name="ps", bufs=8, space="PSUM"))

        # --- small vectors + masks (scalar engine issues these DMAs) ---
        ytb = spool.tile((128, 64), dtf, tag="ytb")
        nc.scalar.dma_start(ytb, yt)
        stb = spool.tile((128, 64), dtf, tag="stb")
        nc.scalar.dma_start(stb, st)
        v = spool.tile((128, 64), dtf, tag="v")
        nc.vector.tensor_sub(v, ytb, stb)
        mask = spool.tile((128, 64), dtf, tag="mask")
        nc.vector.tensor_scalar(mask, v, 0.0, None, op.is_ge)

        ytob = spool.tile((128, 8), dtf, tag="ytob")
        nc.scalar.dma_start(ytob, yto)
        stob = spool.tile((128, 8), dtf, tag="stob")
        nc.scalar.dma_start(stob, sto)
        vo = spool.tile((128, 8), dtf, tag="vo")
        nc.vector.tensor_sub(vo, ytob, stob)
        masko = spool.tile((128, 8), dtf, tag="masko")
        nc.vector.tensor_scalar(masko, vo, 0.0, None, op.is_ge)
        umo = spool.tile((128, 8), dtf, tag="umo")
        nc.vector.tensor_scalar(umo, masko, -1.0, 1.0, op.mult, op.add)

        nqb = spool.tile((128, 4), dtf, tag="nqb")
        nc.scalar.dma_start(nqb, nq)
        xvb = spool.tile((128, 4), dtb, tag="xvb")
        nc.scalar.dma_start(xvb, xv)
        nbb = spool.tile((128, 8), dtb, tag="nbb")
        nc.scalar.dma_start(nbb, nb)

        # --- resident rhs tiles (gpsimd issues these DMAs) -------------
        wag = []
        for g in range(NG1):
            nk = min(G1, KT1 - g * G1)
            t = dpool.tile((128, nk * F), dtb, tag=f"wag{g}")
            nc.gpsimd.dma_start(t, wa[:, g * G1 * F:(g * G1 + nk) * F])
            wag.append(t)

        def dslice(k):
            g, j = k // G1, k % G1
            return wag[g][:, j * F:(j + 1) * F]

        # masked copies of the m-block tiles (dy = mask * wy)
        dm = []
        for j in range(64):
            t = dpool.tile((128, F), dtb, tag=f"dm{j}")
            nc.vector.tensor_scalar_mul(t, dslice(32 + j), mask[:, j:j + 1])
            dm.append(t)

        wosb = spool.tile((128, 8 * F), dtb, tag="wosb")
        nc.gpsimd.dma_start(wosb, wown)
        wm = []
        for t_i in range(8):
            mt = spool.tile((128, F), dtb, tag=f"wm{t_i}")
            nc.vector.tensor_scalar_mul(mt, wosb[:, t_i * F:(t_i + 1) * F],
                                        masko[:, t_i:t_i + 1])
            wm.append(mt)
        xwsb = spool.tile((128, 4 * F), dtb, tag="xwsb")
        nc.gpsimd.dma_start(xwsb, xw)

        def rhs1(k):
            return dslice(k) if (k < 32 or k == 96) else dm[k - 32]

        # --- GEMM1: o1_i (+ Px in col 256) ----------------------------
        ps1 = [pspool.tile((128, F), dtf, tag="ps", name=f"ps1_{m}") for m in range(4)]
        for g in range(NG1):
            nk = min(G1, KT1 - g * G1)
            btt = wpool.tile((128, G1 * NS), dtb, tag="bt",
                             name=f"btt{g}", padded_shape=(128, G1 * NS))
            nc.sync.dma_start(btt[:, :nk * NS],
                              bt[:, g * G1 * NS:(g * G1 + nk) * NS])
            for j in range(nk):
                k = g * G1 + j
                for m in range(4):
                    nc.tensor.matmul(ps1[m],
                                     btt[:, j * NS + m * 128:j * NS + (m + 1) * 128],
                                     rhs1(k), start=(k == 0), stop=(k == KT1 - 1))

        px = spool.tile((128, 4), dtb, tag="px")
        cf = spool.tile((128, 4), dtb, tag="cf")
        for m in range(4):
            ot = opool.tile((128, KP), dtf, tag="o")
            nc.vector.tensor_copy(ot, ps1[m][:, 0:KP])
            nc.scalar.dma_start(out[m * 128:(m + 1) * 128, :], ot)
            nc.vector.tensor_copy(px[:, m:m + 1], ps1[m][:, KP:KP + 1])
            # cf = -(q + 2 Px) = (Px * -2) + (-q)
            nc.vector.scalar_tensor_tensor(cf[:, m:m + 1], ps1[m][:, KP:KP + 1],
                                           -2.0, nqb[:, m:m + 1],
                                           op.mult, op.add)

        # --- GEMM2: o2pre_i = -A_i@dx + b_i dt ------------------------
        ps2 = [pspool.tile((128, F), dtf, tag="ps", name=f"ps2_{t}") for t in range(8)]
        for g in range(NG2):
            nk = min(G2, KT2 - g * G2)
            ctt = cpool.tile((128, G2 * MS), dtb, tag="ct",
                             name=f"ctt{g}", padded_shape=(128, G2 * MS))
            nc.sync.dma_start(ctt[:, :nk * MS],
                              ct[:, g * G2 * MS:(g * G2 + nk) * MS])
            for j in range(nk):
                k = g * G2 + j
                rhs = dslice(k) if k < 32 else dslice(96)
                for t_i in range(8):
                    nc.tensor.matmul(ps2[t_i],
                                     ctt[:, j * MS + t_i * 128:j * MS + (t_i + 1) * 128],
                                     rhs, start=(k == 0), stop=(k == KT2 - 1))
        for t_i in range(8):
            ft = opool.tile((128, KP), dtf, tag="o")
            # f2 = (wy * (1-mask)) + o2pre
            nc.vector.scalar_tensor_tensor(ft, wosb[:, t_i * F:t_i * F + KP],
                                           umo[:, t_i:t_i + 1],
                                           ps2[t_i][:, 0:KP], op.mult, op.add)
            nc.scalar.dma_start(out[NS + t_i * 128:NS + (t_i + 1) * 128, :], ft)

        # --- o3 partial ----------------------------------------------
        pso3 = pspool.tile((1, F), dtf, tag="ps")
        for j in range(4):
            nc.tensor.matmul(pso3, cf[:, j:j + 1], xwsb[:, j * F:(j + 1) * F],
                             start=(j == 0), stop=False)
        for t_i in range(8):
            nc.tensor.matmul(pso3, nbb[:, t_i:t_i + 1], wm[t_i],
                             start=False, stop=(t_i == 7))
        psxx = pspool.tile((1, 1), dtf, tag="ps")
        for j in range(4):
            nc.tensor.matmul(psxx, px[:, j:j + 1], xvb[:, j:j + 1],
                             start=(j == 0), stop=(j == 3))
        o3t = opool.tile((1, KP), dtf, tag="o3t")
        nc.vector.tensor_scalar_mul(o3t, dslice(96)[0:1, 0:KP], psxx[0:1, 0:1])
        o3f = opool.tile((1, KP), dtf, tag="o3f")
        nc.vector.tensor_add(o3f, o3t, pso3[0:1, 0:KP])
        nc.scalar.dma_start(out[NS + MS:NS + MS + 1, :], o3f)

    nc.compile()
    return nc


def _get_nc():
    global _NC_CACHE
    if _NC_CACHE is None:
        _NC_CACHE = _build_nc()
    return _NC_CACHE


def _prep_in_maps(P, A, q, b, x, y, s, W):
    P = np.asarray(P, np.float32)
    A = np.asarray(A, np.float32)
    q = np.asarray(q, np.float32)
    b = np.asarray(b, np.float32)
    x = np.asarray(x, np.float32)
    y = np.asarray(y, np.float32)
    s = np.asarray(s, np.float32)
    W = np.asarray(W, np.float32)

    Pb, Ab = P.astype(BF), A.astype(BF)
    qb, bb, xb, Wb = q.astype(BF), b.astype(BF), x.astype(BF), W.astype(BF)

    wa0 = np.zeros((R1, F), BF)
    wa0[:N + M + 1, :KP] = Wb
    wa0[:N, KP] = xb
    wa = _kt(wa0, KT1, F)
    yt = np.ascontiguousarray(y.reshape(64, 128).T)
    st_ = np.ascontiguousarray(s.reshape(64, 128).T)

    in_maps = []
    for i in range(NC):
        ncol = slice(i * NS, (i + 1) * NS)
        mrow = slice(i * MS, (i + 1) * MS)
        bt0 = np.zeros((R1, NS), BF)
        bt0[:N] = Pb[:, ncol]
        bt0[N:N + M] = Ab[:, ncol]
        bt0[N + M] = qb[ncol]
        ct0 = np.zeros((R2, MS), BF)
        ct0[:N] = (-A[mrow].T).astype(BF)
        ct0[N] = bb[mrow]
        in_maps.append(dict(
            bt=_kt(bt0, KT1, NS), ct=_kt(ct0, KT2, MS), wa=wa,
            yt=yt, st=st_,
            yto=np.ascontiguousarray(yt[:, 8 * i:8 * i + 8]),
            sto=np.ascontiguousarray(st_[:, 8 * i:8 * i + 8]),
            nq=np.ascontiguousarray((-q[ncol]).reshape(4, 128).T),
            xv=np.ascontiguousarray(x[ncol].reshape(4, 128).T.astype(BF)),
            nb=np.ascontiguousarray((-b[mrow]).reshape(8, 128).T.astype(BF)),
            wown=_kt(wa0[N + i * MS:N + (i + 1) * MS], 8, F),
            xw=_kt(wa0[i * NS:(i + 1) * NS], 4, F),
        ))
    return in_maps


def _assemble(results):
    Fo = np.empty((N + M + 1, KP), np.float32)
    o3 = np.zeros((KP,), np.float32)
    for i in range(NC):
        o = np.asarray(results[i]["out"], np.float32)
        Fo[i * NS:(i + 1) * NS] = o[:NS]
        Fo[N + i * MS:N + (i + 1) * MS] = o[NS:NS + MS]
        o3 += o[NS + MS]
    Fo[N + M] = o3
    return Fo


def _run_sharded(inputs, trace=False, trace_kwargs=None):
    from concourse import bass_utils
    nc = _get_nc()
    in_maps = _prep_in_maps(**inputs)
    res = bass_utils.run_bass_kernel_spmd(
        nc, in_maps, core_ids=list(range(NC)), trace=trace,
        **(trace_kwargs or {}))
    return _assemble(res.results), res


def kernel(**inputs) -> np.ndarray:
    out, _ = _run_sharded(inputs, trace=False)
    return out
